# revision 41
# baseline (speedup 1.0000x reference)
"""Fused QKV + multi-head attention kernel for Trainium2 (Bass/Tile), 8-core SPMD.

Problem: x[4, 2048, 1024] -> qkv = x @ W_qkv + b_qkv -> 16-head attention -> out[4, 2048, 1024].

Sharding (DP x TP): core c handles batch c//2 and head-group c%2 (8 of 16 heads),
so each core runs the qkv projection for its batch restricted to its heads'
columns of W_qkv, plus full attention for its 8 heads. No cross-core comm.

Per-core design (bf16 matmuls, fp32 accumulation). The PE matmul stream
(~250us of array streaming) is the floor; the exp of the score matrix
(33.5M elems) is spread across three engines so it never paces the kernel:
ScalarE does ~10/16 tiles per phase exactly (1.11us/tile), the otherwise-idle
GpSimd(Pool) engine does ~5/16 as a Schraudolph fast exp (single [128,1024]
tensor_scalar, ~0.92us/tile), and in pair 3 the VectorE picks up 2 more via
the idle ps banks. Key points:

 - all exp tiles read the pss PSUM double-buffer directly; with ScalarE
   relieved, the pss recycle always outruns the PE slot cadence, so score
   matmuls are emitted in batches of two jj in ALL pairs (PE reconfigures
   its row-group split half as often, ~100ns/switch).
 - Schraudolph fast exp: bits = round(A*s + B) written as int16 IS the bf16
   bit pattern of exp(s/8), fed to the AV matmul via a dtype bitcast
   (~2% RMS on those tiles, ~9e-3 end-to-end with the chosen tile sets).
   Pair 0 keeps exact exp everywhere (its phase is PE-bound with ScalarE
   slack), pair 3 offloads the most (no prefetch duty -> lightest PE).
 - x DMA is staged per 512-token group; group 0 is issued per k-tile so the
   first qk chains drain at DMA-arrival pace, groups 1-3 are one coalesced
   3D DMA each (frees ~14us of sync-queue issue time); the pairs-1-3 W
   columns are DMA'd from the weave after the x stream finishes.
 - all projection work (q/k chains just-in-time, v chains, next-pair
   prefetch at ~1 matmul/slot) is woven into the per-jj slack of the
   attention loop at 2-matmul granularity, keyed to x-group arrival order.
 - AV matmuls sit in a global in-order queue, gated on their v-chain having
   been emitted and delayed >=1 slot (2 for fast-exp tiles) so exp latency
   never lands between consecutive score matmuls in the in-order PE stream.
 - v-bias is folded into the v projection (exact: sum_k softmax * (v+bv) =
   out + bv); scores^T = k^T.T @ q^T with heads packed in partition halves
   (row-tiled concurrent matmul pairs); AV rides a 65th ones-row in v to
   accumulate the softmax denominator.
 - steady-state normalization: the reciprocal is computed across 128 lanes
   via the SBUF->SBUF transpose DMA + DRAM-bounce broadcast, pipelined
   across slots; both heads' normalized outputs land in one [128,512] tile
   and ship as a single DMA. The FINAL (p,ig) instead uses a fast path with
   no DRAM bounce: reciprocal of the ones-row, broadcast across partitions
   by a tiny ones^T @ rcp matmul on the (then idle) PE, cutting the tail
   from ~14us to ~7us.
"""

import sys

sys.path.insert(0, "/opt/trn_rl_repo")

import numpy as np
import ml_dtypes

T = 2048
D = 1024
NH_LOCAL = 8  # heads per core
HS = 64
WCOLS = NH_LOCAL * 3 * HS  # 1536
VCOLS = NH_LOCAL * HS  # 512
KT = D // 128  # 8 contraction tiles
PAIRS = NH_LOCAL // 2  # 4
IG = T // 512  # 4 query groups
JT = T // 128  # 16 key tiles

# Schraudolph fast-exp constants: int16 bits of bf16(exp(s*0.125)).
# bits = round(A*s + B); A folds the 1/sqrt(hs) score scale.
FEXP_A = 0.125 * 128.0 / float(np.log(2.0))
FEXP_B = 16250.0


def pss_fexp_set(p):
    # jj slots whose exp runs on VectorE (Schraudolph, single [128,1024]
    # tensor_scalar) reading the pss banks directly. Pair 0 stays exact: its
    # phase is PE-bound (all projection weave) with plenty of ScalarE slack,
    # so spending error budget there buys nothing. GpSimd cannot touch PSUM
    # on TRN2, so VectorE is the only offload engine for PSUM-resident
    # scores; its queue is kept clear by pushing all-SBUF work to GpSimd.
    # Pairs 2/3 run as interleaved ig-phases with no chain filler, so they
    # carry the most offload.
    return {0: (), 1: (7, 13), 2: (5, 11), 3: (5, 11)}[p]


def ps_fexp_set(p):
    # fast-exp slots on VectorE routed through the ps banks, which are
    # idle during the interleaved pair-2/3 phases (their qk chains were all
    # prefetched during pair 1): the slow DVE exp never holds a pss bank.
    return (1, 7, 13) if p in (2, 3) else ()


def ps_route_set(p):
    # jj whose scores land in the ps bank pair — the middle element of each
    # 3-jj batch in the interleaved phases. The ones not in ps_fexp_set are
    # exp'd by ScalarE as two [128,512] ops.
    return (1, 4, 7, 10, 13) if p in (2, 3) else ()


def jj_batches(p):
    # score-matmul batches: all jj of a batch are emitted back-to-back so
    # the PE enters/leaves the row-grouped mode once per batch (~200ns per
    # switch pair). Pairs 0/1 rotate scores over the two pss tiles ->
    # batches of 2; the interleaved pairs 2/3 also use the (otherwise idle)
    # ps bank pair as a third score slot -> batches of 3, which both cuts
    # switch pairs 8->6 per phase and gives every score bank a 3-jj reuse
    # distance so the exp engines never gate the PE.
    if p in (2, 3):
        return ((0, 1, 2), (3, 4, 5), (6, 7, 8), (9, 10, 11), (12, 13, 14), (15,))
    return tuple((j, j + 1) for j in range(0, JT, 2))


_CACHE: dict = {}


def _emit(tc, x_d, w_d, bqk_d, bv_d, out_d):
    import concourse.bass as bass
    from concourse import mybir
    from contextlib import ExitStack

    nc = tc.nc
    f32 = mybir.dt.float32
    bf16 = mybir.dt.bfloat16
    i16 = mybir.dt.int16
    Exp = mybir.ActivationFunctionType.Exp
    MULT = mybir.AluOpType.mult
    ADD = mybir.AluOpType.add

    E_BUFS = 22
    ctx = ExitStack()
    pers = ctx.enter_context(tc.tile_pool(name="pers", bufs=1))
    qk_pool = ctx.enter_context(tc.tile_pool(name="qk", bufs=PAIRS))
    e_pool = ctx.enter_context(tc.tile_pool(name="epool", bufs=E_BUFS))
    e16_pool = ctx.enter_context(tc.tile_pool(name="e16pool", bufs=8))
    o_pool = ctx.enter_context(tc.tile_pool(name="opool", bufs=4))
    ot_pool = ctx.enter_context(tc.tile_pool(name="otpool", bufs=4))
    sm_pool = ctx.enter_context(tc.tile_pool(name="smpool", bufs=4))
    dr_pool = ctx.enter_context(tc.tile_pool(name="drpool", bufs=4, space="DRAM"))
    # PSUM: 8 banks = ps 2 (proj) + pss 2x2 (scores/exp staging) + po 2 (AV).
    ps_pool = ctx.enter_context(tc.tile_pool(name="pspool", bufs=2, space="PSUM"))
    pss_pool = ctx.enter_context(tc.tile_pool(name="psspool", bufs=2, space="PSUM"))
    po_pool = ctx.enter_context(tc.tile_pool(name="popool", bufs=2, space="PSUM"))

    # ---- ACT exp-table preload: runs during the input DMA phase ----
    warm = sm_pool.tile([1, 8], f32, tag="warm")
    nc.vector.memset(warm, 0.0)
    nc.scalar.activation(warm, warm, Exp)

    # ---- staged input DMA: x group 0 + biases first, then the rest ----
    xt_sb = pers.tile([128, KT, T], bf16)  # xT[k, tok] per k-tile
    bqk_sb = pers.tile([128, PAIRS, 2], f32)  # [part, pair, q/k] per-partition bias
    # x group 0 per k-tile on the sync queue (startup chains drain at DMA
    # arrival pace); groups 1-3 as one coalesced 3D DMA each; bqk alone on
    # the scalar queue (each scalar-queue issue costs ~600ns of ACT time).
    nc.scalar.dma_start(bqk_sb, bqk_d)
    # group 0 per k-tile (startup chains drain at per-tile arrival pace);
    # groups 1-3 in half-group chunks: 6 issues instead of 24 drains the
    # sync issue queue ~15us earlier, so the k(g2)/k(g3) chains in the
    # pair-0 weave never wait on x arrival.
    for kk in range(KT):
        nc.sync.dma_start(
            xt_sb[:, kk, 0:512], x_d[kk * 128 : (kk + 1) * 128, 0:512]
        )
    x_r = x_d.rearrange("(kk p) t -> p kk t", p=128)
    for g in range(1, IG):
        for half in range(2):
            k0 = half * 4
            nc.sync.dma_start(
                xt_sb[:, k0 : k0 + 4, g * 512 : (g + 1) * 512],
                x_r[:, k0 : k0 + 4, g * 512 : (g + 1) * 512],
            )

    # w: pair-0 qk columns + v-bias + v columns now; the remaining qk columns
    # (pairs 1-3, 1.5MB) are issued from the weave once the x stream is done,
    # so they never compete with x for HBM bandwidth.
    w_sb = pers.tile([128, KT, WCOLS], bf16)
    bv_bc = pers.tile([128, VCOLS], f32)  # v-bias broadcast across token rows
    for kk in range(KT):
        nc.gpsimd.dma_start(w_sb[:, kk, 0:256], w_d[kk * 128 : (kk + 1) * 128, 0:256])
    nc.gpsimd.dma_start(bv_bc, bv_d.partition_broadcast(128))
    # v columns on the scalar queue: it is idle after bqk (ScalarE's first
    # exp is not until ~14us) and the gpsimd queue is busy issuing the
    # qk columns — this lands v ~5us earlier so the first AVs are not
    # gated on the v projection.
    for kk in range(KT):
        nc.scalar.dma_start(
            w_sb[:, kk, 1024:WCOLS], w_d[kk * 128 : (kk + 1) * 128, 1024:WCOLS]
        )

    def w_rest_dma_units():
        # sync queue: idle once the x stream is done, and ScalarE/Pool now
        # carry exp work so their queues must stay clear.
        def half(lo_kk):
            def emit():
                for kk in range(lo_kk, lo_kk + 4):
                    nc.sync.dma_start(
                        w_sb[:, kk, 256:1024],
                        w_d[kk * 128 : (kk + 1) * 128, 256:1024],
                    )

            return emit

        return [half(0), half(4)]

    # ---- v ones-columns ----
    v_sb = pers.tile([128, JT, NH_LOCAL, HS + 1], bf16)
    nc.vector.memset(v_sb[:, :, :, HS : HS + 1], 1.0)
    # ones row for the tail-norm partition broadcast (ones^T @ rcp on PE);
    # bf16 so the broadcast matmul streams 1 row/cycle (1.0 and 1/den are
    # fine in bf16 here: it scales only the final ig's normalization)
    ones_sb = pers.tile([1, HS], bf16)
    nc.vector.memset(ones_sb, 1.0)

    # ================= emission engine =================
    # Work units are closures emitting ~2 matmuls (or a small DVE op) each.
    # They are pumped into the per-jj slack of the attention loop.

    def qk_chain_units(p, g, qk):
        # q^T or k^T for pair p, token group g -> qk_tiles[p][:, qk, g*512:]
        # psum partitions 0-63 <- head 2p, 64-127 <- head 2p+1
        st = {}

        def quarter(q4):
            def emit():
                if q4 == 0:
                    st["ps"] = ps_pool.tile([128, 512], f32, tag="ps", name="psqk")
                ps = st["ps"]
                c0 = (p * 2 + qk) * 128
                for kk in range(2 * q4, 2 * q4 + 2):
                    nc.tensor.matmul(
                        ps,
                        w_sb[:, kk, c0 : c0 + 128],
                        xt_sb[:, kk, g * 512 : (g + 1) * 512],
                        start=(kk == 0),
                        stop=(kk == KT - 1),
                    )
                if q4 == 3:
                    nc.vector.tensor_scalar_add(
                        qk_tiles[p][:, qk, g * 512 : (g + 1) * 512],
                        ps,
                        bqk_sb[:, p, qk : qk + 1],
                    )

            return emit

        return [quarter(q4) for q4 in range(4)]

    def v_chain_units(tt):
        # v[tok tile tt, all heads] = xT.T @ Wv, plus folded v-bias
        st = {}

        def quarter(q4):
            def emit():
                if q4 == 0:
                    st["ps"] = ps_pool.tile([128, 512], f32, tag="ps", name="psv")
                ps = st["ps"]
                for kk in range(2 * q4, 2 * q4 + 2):
                    nc.tensor.matmul(
                        ps,
                        xt_sb[:, kk, tt * 128 : (tt + 1) * 128],
                        w_sb[:, kk, 1024:1536],
                        start=(kk == 0),
                        stop=(kk == KT - 1),
                    )
                if q4 == 3:
                    nc.vector.tensor_tensor(
                        v_sb[:, tt, :, 0:HS],
                        ps.rearrange("p (h c) -> p h c", c=HS),
                        bv_bc.rearrange("p (h c) -> p h c", c=HS),
                        op=ADD,
                    )
                    v_emitted[tt] = True

            return emit

        return [quarter(q4) for q4 in range(4)]

    qk_tiles = [
        qk_pool.tile([128, 2, T], bf16, tag="qkt", name=f"qkt{_p}") for _p in range(PAIRS)
    ]
    v_emitted = [False] * JT

    # Global AV queue: (slot, p, ig, jj, e_ap) emitted strictly in order, each
    # gated on its v tile having been emitted AND at least one slot old (so
    # the exp latency never sits between consecutive scores in the PE stream).
    av_queue = []
    norm_jobs = []  # normalization pipeline jobs (step >= 1; step 0 runs inline)
    po_live = {}
    slot_ctr = [0]

    def pump_avs(limit=3, flush=False):
        n = 0
        while av_queue and n < limit:
            sl, p, ig, jj, e_bf = av_queue[0]
            if not v_emitted[jj]:
                break
            # two slots of delay for every tile: with scores batched two jj
            # at a time, AV(jj+1) precedes the next scores batch in the
            # in-order PE stream, and its exp lands ~2.3us after the batch
            # (two back-to-back ACT exps); one slot of slack is not enough
            # and the exp latency would pace the whole PE stream. Fast-exp
            # tiles get one more slot: the DVE runs behind a deeper queue.
            delay = 3 if (jj in pss_fexp_set(p) or jj in ps_fexp_set(p)) else 2
            if not flush and sl + delay > slot_ctr[0]:
                break
            if (p, ig) not in po_live:
                po_live[(p, ig)] = [
                    po_pool.tile([65, 512], f32, tag="po", name=f"po{_h}")
                    for _h in range(2)
                ]
            po = po_live[(p, ig)]
            av_queue.pop(0)
            for h in range(2):
                nc.tensor.matmul(
                    po[h],
                    v_sb[:, jj, 2 * p + h, :],
                    e_bf[:, h * 512 : (h + 1) * 512],
                    start=(jj == 0),
                    stop=(jj == JT - 1),
                )
            n += 1
            if jj == JT - 1:
                # Free the po bank pair NOW (copy to SBUF) so the next ig's
                # AVs — emitted later — see the reader before the reuse.
                po = po_live.pop((p, ig))
                o_t = [
                    o_pool.tile([65, 512], f32, tag="o", name=f"ot{_h}")
                    for _h in range(2)
                ]
                # the next phase's first AV reuses these po banks: in the
                # interleaved pair-2/3 phases split the two copies across
                # ScalarE and VectorE so both banks free ~0.7us sooner
                if p >= 2:
                    nc.scalar.activation(
                        o_t[0], po[0], mybir.ActivationFunctionType.Copy
                    )
                    nc.vector.tensor_copy(o_t[1], po[1])
                else:
                    for h in range(2):
                        nc.vector.tensor_copy(o_t[h], po[h])
                norm_jobs.append({"p": p, "ig": ig, "o": o_t, "step": 1})

    # Normalization pipeline: remaining steps spread across subsequent slots
    # so the DRAM-bounce latency never blocks the in-order DVE stream.
    def norm_tail_fast(job):
        # Final (p,ig): PE is idle, so broadcast 1/den across partitions with
        # a tiny ones^T @ rcp matmul instead of the 3-DMA DRAM bounce.
        p, ig = job["p"], job["ig"]
        den_b = [
            sm_pool.tile([1, 512], bf16, tag="rcp", name=f"denb{_h}")
            for _h in range(2)
        ]
        bc = [
            po_pool.tile([64, 512], f32, tag="po", name=f"bc{_h}") for _h in range(2)
        ]
        rb = [
            sm_pool.tile([64, 512], f32, tag="denbc", name=f"rb{_h}")
            for _h in range(2)
        ]
        ot2 = ot_pool.tile([128, 512], f32, tag="ot")
        for h in range(2):
            # broadcast den (not 1/den) across partitions — the reciprocal
            # then runs on a [64,512] tile where the DVE is fast, instead of
            # on a single-partition row (~6.5ns/elem, 3.3us)
            nc.vector.tensor_copy(den_b[h], job["o"][h][64:65, :])
            nc.tensor.matmul(bc[h], ones_sb, den_b[h], start=True, stop=True)
            # full RECIPROCAL is ~6.5ns/elem (3.3us here); the NR-seeded
            # approx (18 bits; den is a benign ~1e2..1e3 positive value)
            # runs at ~1 elem/lane/cycle on this multi-partition tile
            nc.vector.reciprocal_approx_fast(rb[h], bc[h])
            nc.vector.tensor_tensor(
                ot2[h * 64 : (h + 1) * 64, :], job["o"][h][0:64, :], rb[h], op=MULT
            )
        nc.sync.dma_start(
            out_d[128 * p : 128 * (p + 1), ig * 512 : (ig + 1) * 512], ot2
        )

    def norm_step():
        if not norm_jobs:
            return
        job = norm_jobs[0]
        p, ig, step = job["p"], job["ig"], job["step"]
        if step == 1 and (p, ig) == (PAIRS - 1, IG - 1):
            norm_tail_fast(job)
            norm_jobs.pop(0)
            return
        if step == 1:
            # den row -> [128, 4] transposed layout (SBUF->SBUF DMA)
            rct = sm_pool.tile([128, 8], f32, tag="rct")
            for h in range(2):
                nc.sync.dma_start(rct[:, h * 4 : (h + 1) * 4], job["o"][h][64:65, :])
            job["rct"] = rct
        elif step == 0:
            raise AssertionError("step 0 runs inline in pump_avs")
        elif step == 2:
            rcp = sm_pool.tile([128, 8], f32, tag="rcp")
            nc.vector.reciprocal(rcp, job["rct"])
            job["rcp"] = rcp
        elif step == 3:
            rcd = dr_pool.tile([2, 512], f32, tag="rcd")
            for h in range(2):
                nc.sync.dma_start(rcd[h], job["rcp"][:, h * 4 : (h + 1) * 4])
            job["rcd"] = rcd
        elif step == 4:
            job["dbc"] = []
            for h in range(2):
                den_bc = sm_pool.tile([64, 512], f32, tag="denbc", name=f"dbc{h}")
                nc.gpsimd.dma_start(den_bc, job["rcd"][h].partition_broadcast(64))
                job["dbc"].append(den_bc)
        elif step == 5:
            # all-SBUF op: runs on GpSimd to keep the VectorE queue clear
            # for the fast-exp tiles that pace the pss recycle
            ot2 = ot_pool.tile([128, 512], f32, tag="ot")
            for h in range(2):
                nc.gpsimd.tensor_tensor(
                    ot2[h * 64 : (h + 1) * 64, :],
                    job["o"][h][0:64, :],
                    job["dbc"][h],
                    op=MULT,
                )
            nc.sync.dma_start(
                out_d[128 * p : 128 * (p + 1), ig * 512 : (ig + 1) * 512], ot2
            )
            norm_jobs.pop(0)
            return
        job["step"] += 1

    # ---- startup: q/k chains for pair 0, group 0, interleaved at k-tile
    # granularity so each matmul runs as soon as its x k-tile DMA lands ----
    ps_q = ps_pool.tile([128, 512], f32, tag="ps", name="psq0")
    ps_k = ps_pool.tile([128, 512], f32, tag="ps", name="psk0")
    for kk in range(KT):
        for qk, ps in ((0, ps_q), (1, ps_k)):
            nc.tensor.matmul(
                ps,
                w_sb[:, kk, qk * 128 : (qk + 1) * 128],
                xt_sb[:, kk, 0:512],
                start=(kk == 0),
                stop=(kk == KT - 1),
            )
    for qk, ps in ((0, ps_q), (1, ps_k)):
        nc.vector.tensor_scalar_add(
            qk_tiles[0][:, qk, 0:512], ps, bqk_sb[:, 0, qk : qk + 1]
        )

    # ---- build weave schedules ----
    # pair 0: k chains first (hard scores deadlines at jj=4g; x groups arrive
    # progressively so these are also arrival-ordered), then q(g1), then the
    # v chains (AVs defer via the queue), with q(g2)/q(g3) at the end (their
    # deadlines are late). The w remainder DMA rides early in the stream,
    # once x is (nearly) done. Later pairs: chains prefetched in prior phase.
    pair_units = {p: [] for p in range(PAIRS)}
    u0 = pair_units[0]
    # k chains are hard in-order deadlines (scores jj=4g at slot 4g) AND
    # gate on x-group arrival, so each is placed at the slot its group
    # lands; v chains fill between (their AVs defer via the queue and the
    # deep e pool absorbs the backlog); q(g1) before ig1.
    u0.extend(qk_chain_units(0, 1, 1))  # k(g1): x g1 lands ~slot 1
    for tt in range(2):
        u0.extend(v_chain_units(tt))
    u0.extend(qk_chain_units(0, 2, 1))  # k(g2): x g2 lands ~slot 3-4
    for tt in range(2, 4):
        u0.extend(v_chain_units(tt))
    u0.extend(qk_chain_units(0, 3, 1))  # k(g3): x g3 lands ~slot 6-7
    u0.extend(qk_chain_units(0, 1, 0))  # q(g1) before ig1
    for tt in range(4, 10):
        u0.extend(v_chain_units(tt))
    u0.extend(w_rest_dma_units())  # issue pairs 1-3 W columns (HBM now free)
    for tt in range(10, JT):
        u0.extend(v_chain_units(tt))
    u0.extend(qk_chain_units(0, 2, 0))  # q(g2) before ig2
    u0.extend(qk_chain_units(0, 3, 0))  # q(g3) before ig3
    for p in range(1, PAIRS):
        for g in range(IG):
            for qk in range(2):
                pair_units[p].extend(qk_chain_units(p, g, qk))

    # interleave: during each ig-phase, pump the leftover units of its own
    # schedule (front-loaded) plus the next schedule's prefetch units
    # (~1 matmul/slot). jj are processed in batches of 2 in all phases so
    # the PE switches between row-grouped score matmuls and full-array
    # AV/proj matmuls half as often (each switch costs ~100ns of array
    # drain). Pairs 2 and 3 run as interleaved ig-phases — (2,0),(3,0),
    # (2,1),(3,1),... — so their combined exp demand is smoothed across
    # ScalarE/VectorE instead of pair 3 (no chain filler, lightest PE)
    # being exp-bound on its own.
    def attention_phase(p, ig, own, nxt):
        pss_fexp = pss_fexp_set(p)
        ps_fexp = ps_fexp_set(p)
        ps_route = ps_route_set(p)
        qk_t = qk_tiles[p]

        def scores_mm(dst, jj, h):
            base = 64 * h
            nc.tensor.matmul(
                dst,
                qk_t[base : base + 64, 1, jj * 128 : (jj + 1) * 128],
                qk_t[base : base + 64, 0, ig * 512 : (ig + 1) * 512],
                start=True,
                stop=True,
            )

        def emit_scores(jj):
            # scores^T for both heads (row-tiled concurrent pair)
            pss = pss_pool.tile([128, 1024], f32, tag="pss")
            for h in range(2):
                scores_mm(pss[:, h * 512 : (h + 1) * 512], jj, h)
            return pss

        for batch in jj_batches(p):
            # emit the whole batch's score matmuls back-to-back, then the
            # exp ops in jj order
            staged = []
            for j2 in batch:
                if j2 in ps_route:
                    psf = [
                        ps_pool.tile([128, 512], f32, tag="ps", name=f"psf{_h}")
                        for _h in range(2)
                    ]
                    for h in range(2):
                        scores_mm(psf[h], j2, h)
                    staged.append(("f" if j2 in ps_fexp else "pa", j2, psf))
                else:
                    staged.append(
                        ("v" if j2 in pss_fexp else "a", j2, emit_scores(j2))
                    )
            for o in staged:
                cls, j2, src = o
                if cls == "a":
                    e_t = e_pool.tile([128, 1024], bf16, tag="e")
                    nc.scalar.activation(e_t, src, Exp, scale=0.125)
                    av_queue.append((slot_ctr[0], p, ig, j2, e_t))
                elif cls == "pa":
                    # ps-routed scores exp'd on ScalarE as two half tiles
                    e_t = e_pool.tile([128, 1024], bf16, tag="e")
                    for h in range(2):
                        nc.scalar.activation(
                            e_t[:, h * 512 : (h + 1) * 512], src[h], Exp, scale=0.125
                        )
                    av_queue.append((slot_ctr[0], p, ig, j2, e_t))
                elif cls == "v":
                    # single [128,1024] op straight off the pss bank
                    e16 = e16_pool.tile([128, 1024], i16, tag="e16")
                    nc.vector.tensor_scalar(
                        e16, src, FEXP_A, FEXP_B, op0=MULT, op1=ADD
                    )
                    av_queue.append((slot_ctr[0], p, ig, j2, e16.bitcast(bf16)))
                else:
                    e16 = e16_pool.tile([128, 1024], i16, tag="e16")
                    for h in range(2):
                        nc.vector.tensor_scalar(
                            e16[:, h * 512 : (h + 1) * 512],
                            src[h],
                            FEXP_A,
                            FEXP_B,
                            op0=MULT,
                            op1=ADD,
                        )
                    av_queue.append((slot_ctr[0], p, ig, j2, e16.bitcast(bf16)))
            n_f = sum(
                1
                for a in av_queue
                if a[3] in pss_fexp_set(a[1]) or a[3] in ps_fexp_set(a[1])
            )
            assert n_f <= 6, "e16 backlog exceeds pool depth"
            for jj in batch:
                # emission-order safety: a pool slot must not be re-tiled
                # before its deferred AV reader has been emitted
                assert len(av_queue) <= E_BUFS - 2, (
                    f"AV backlog {len(av_queue)} exceeds e pool depth"
                )
                # weave proj units into the slack of this slot; next-schedule
                # prefetch is paced at ~1 unit (2 matmuls) per slot
                if own:
                    for _ in range(min(3, len(own))):
                        own.pop(0)()
                elif nxt:
                    nxt.pop(0)()
                pump_avs(limit=3)
                norm_step()
                slot_ctr[0] += 1

    # pairs 2 and 3 share one prefetch schedule (built during pair 1) and
    # run as interleaved ig-phases
    merged23 = [u for ab in zip(pair_units[2], pair_units[3]) for u in ab]
    phases = (
        [(0, g) for g in range(IG)]
        + [(1, g) for g in range(IG)]
        + [(q, g) for g in range(IG) for q in (2, 3)]
    )
    for p, ig in phases:
        if p == 0:
            own, nxt = pair_units[0], pair_units[1]
        elif p == 1:
            own, nxt = pair_units[1], merged23
        else:
            own, nxt = merged23, []
        attention_phase(p, ig, own, nxt)
    # flush stragglers
    while av_queue:
        pump_avs(flush=True)
    while norm_jobs:
        norm_step()
    ctx.close()


def _build():
    import concourse.tile as tile
    from concourse import bacc, mybir

    f32 = mybir.dt.float32
    nc = bacc.Bacc("TRN2", target_bir_lowering=False, debug=False, num_devices=8)
    x_d = nc.dram_tensor("x", [D, T], mybir.dt.bfloat16, kind="ExternalInput").ap()
    w_d = nc.dram_tensor("w", [D, WCOLS], mybir.dt.bfloat16, kind="ExternalInput").ap()
    bqk_d = nc.dram_tensor("bqk", [128, PAIRS, 2], f32, kind="ExternalInput").ap()
    bv_d = nc.dram_tensor("bv", [VCOLS], f32, kind="ExternalInput").ap()
    out_d = nc.dram_tensor("out", [VCOLS, T], f32, kind="ExternalOutput").ap()
    with tile.TileContext(nc) as tc:
        _emit(tc, x_d, w_d, bqk_d, bv_d, out_d)
    nc.compile()
    return nc


def get_nc():
    if "nc" not in _CACHE:
        _CACHE["nc"] = _build()
    return _CACHE["nc"]


def make_in_maps(x, W_qkv, b_qkv):
    """Shard full inputs into 8 per-core input maps."""
    x = np.asarray(x, dtype=np.float32)
    W_qkv = np.asarray(W_qkv, dtype=np.float32)
    b_qkv = np.asarray(b_qkv, dtype=np.float32)
    in_maps = []
    for c in range(8):
        b, half = divmod(c, 2)
        w_c = W_qkv[:, half * WCOLS : (half + 1) * WCOLS]
        b_c = b_qkv[half * WCOLS : (half + 1) * WCOLS]
        # permute columns: paired q/k blocks first, then v cols in head order
        w3 = w_c.reshape(D, NH_LOCAL, 3, HS)
        blocks = []
        for p in range(PAIRS):
            for qk in range(2):
                blocks.append(w3[:, 2 * p, qk, :])
                blocks.append(w3[:, 2 * p + 1, qk, :])
        for h in range(NH_LOCAL):
            blocks.append(w3[:, h, 2, :])
        w_c = np.concatenate(blocks, axis=1).astype(ml_dtypes.bfloat16)
        # per-partition qk bias: partitions 0-63 <- head 2p, 64-127 <- head 2p+1
        bqk = np.zeros((128, PAIRS, 2), dtype=np.float32)
        for p in range(PAIRS):
            for qk in range(2):
                bqk[0:64, p, qk] = b_c[(2 * p) * 192 + qk * 64 : (2 * p) * 192 + (qk + 1) * 64]
                bqk[64:128, p, qk] = b_c[(2 * p + 1) * 192 + qk * 64 : (2 * p + 1) * 192 + (qk + 1) * 64]
        bv = np.ascontiguousarray(
            b_c.reshape(NH_LOCAL, 3, HS)[:, 2, :].reshape(VCOLS)
        )
        in_maps.append(
            {
                "x": np.ascontiguousarray(x[b].T).astype(ml_dtypes.bfloat16),
                "w": w_c,
                "bqk": bqk,
                "bv": bv,
            }
        )
    return in_maps


def assemble_output(results):
    out = np.zeros((4, T, D), dtype=np.float32)
    for c in range(8):
        b, half = divmod(c, 2)
        out[b, :, half * VCOLS : (half + 1) * VCOLS] = results[c]["out"].T
    return out


def kernel(x, W_qkv, b_qkv):
    from concourse.bass_utils import run_bass_kernel_spmd

    nc = get_nc()
    in_maps = make_in_maps(x, W_qkv, b_qkv)
    res = run_bass_kernel_spmd(nc, in_maps, core_ids=list(range(8)))
    return assemble_output(res.results)


if __name__ == "__main__":
    xs = np.random.randn(4, T, D).astype(np.float32)
    Ws = (np.random.randn(D, 3 * D) / 32.0).astype(np.float32)
    bs = (np.random.randn(3 * D) * 0.02).astype(np.float32)
    o = kernel(xs, Ws, bs)
    print(o.shape, o.dtype)


# revision 42
# speedup vs baseline: 1.0114x; 1.0114x over previous
"""Fused QKV + multi-head attention kernel for Trainium2 (Bass/Tile), 8-core SPMD.

Problem: x[4, 2048, 1024] -> qkv = x @ W_qkv + b_qkv -> 16-head attention -> out[4, 2048, 1024].

Sharding (DP x TP): core c handles batch c//2 and head-group c%2 (8 of 16 heads),
so each core runs the qkv projection for its batch restricted to its heads'
columns of W_qkv, plus full attention for its 8 heads. No cross-core comm.

Per-core design (bf16 matmuls, fp32 accumulation). The PE matmul stream
(~250us of array streaming) is the floor; the exp of the score matrix
(33.5M elems) is spread across three engines so it never paces the kernel:
ScalarE does ~10/16 tiles per phase exactly (1.11us/tile), the otherwise-idle
GpSimd(Pool) engine does ~5/16 as a Schraudolph fast exp (single [128,1024]
tensor_scalar, ~0.92us/tile), and in pair 3 the VectorE picks up 2 more via
the idle ps banks. Key points:

 - all exp tiles read the pss PSUM double-buffer directly; with ScalarE
   relieved, the pss recycle always outruns the PE slot cadence, so score
   matmuls are emitted in batches of two jj in ALL pairs (PE reconfigures
   its row-group split half as often, ~100ns/switch).
 - Schraudolph fast exp: bits = round(A*s + B) written as int16 IS the bf16
   bit pattern of exp(s/8), fed to the AV matmul via a dtype bitcast
   (~2% RMS on those tiles, ~9e-3 end-to-end with the chosen tile sets).
   Pair 0 keeps exact exp everywhere (its phase is PE-bound with ScalarE
   slack), pair 3 offloads the most (no prefetch duty -> lightest PE).
 - x DMA is staged per 512-token group; group 0 is issued per k-tile so the
   first qk chains drain at DMA-arrival pace, groups 1-3 are one coalesced
   3D DMA each (frees ~14us of sync-queue issue time); the pairs-1-3 W
   columns are DMA'd from the weave after the x stream finishes.
 - all projection work (q/k chains just-in-time, v chains, next-pair
   prefetch at ~1 matmul/slot) is woven into the per-jj slack of the
   attention loop at 2-matmul granularity, keyed to x-group arrival order.
 - AV matmuls sit in a global in-order queue, gated on their v-chain having
   been emitted and delayed >=1 slot (2 for fast-exp tiles) so exp latency
   never lands between consecutive score matmuls in the in-order PE stream.
 - v-bias is folded into the v projection (exact: sum_k softmax * (v+bv) =
   out + bv); scores^T = k^T.T @ q^T with heads packed in partition halves
   (row-tiled concurrent matmul pairs); AV rides a 65th ones-row in v to
   accumulate the softmax denominator.
 - steady-state normalization: the reciprocal is computed across 128 lanes
   via the SBUF->SBUF transpose DMA + DRAM-bounce broadcast, pipelined
   across slots; both heads' normalized outputs land in one [128,512] tile
   and ship as a single DMA. The FINAL (p,ig) instead uses a fast path with
   no DRAM bounce: reciprocal of the ones-row, broadcast across partitions
   by a tiny ones^T @ rcp matmul on the (then idle) PE, cutting the tail
   from ~14us to ~7us.
"""

import sys

sys.path.insert(0, "/opt/trn_rl_repo")

import numpy as np
import ml_dtypes

T = 2048
D = 1024
NH_LOCAL = 8  # heads per core
HS = 64
WCOLS = NH_LOCAL * 3 * HS  # 1536
VCOLS = NH_LOCAL * HS  # 512
KT = D // 128  # 8 contraction tiles
PAIRS = NH_LOCAL // 2  # 4
IG = T // 512  # 4 query groups
JT = T // 128  # 16 key tiles

# Schraudolph fast-exp constants: int16 bits of bf16(exp(s*0.125)).
# bits = round(A*s + B); A folds the 1/sqrt(hs) score scale.
FEXP_A = 0.125 * 128.0 / float(np.log(2.0))
FEXP_B = 16250.0


def pss_fexp_set(p):
    # jj slots whose exp runs on VectorE (Schraudolph, single [128,1024]
    # tensor_scalar) reading the pss banks directly. Pair 0 stays exact: its
    # phase is PE-bound (all projection weave) with plenty of ScalarE slack,
    # so spending error budget there buys nothing. GpSimd cannot touch PSUM
    # on TRN2, so VectorE is the only offload engine for PSUM-resident
    # scores; its queue is kept clear by pushing all-SBUF work to GpSimd.
    # Pairs 2/3 run as interleaved ig-phases with no chain filler, so they
    # carry the most offload; the {3,9,15} staggering gives every DVE-read
    # pss bank a 3-jj reuse distance.
    return {0: (), 1: (7, 13), 2: (3, 9, 15), 3: (3, 9, 15)}[p]


def ps_fexp_set(p):
    # extra fast-exp slots on VectorE routed through the ps banks, which are
    # idle during the interleaved pair-2/3 phases (their qk chains were all
    # prefetched during pair 1): the slow DVE exp never holds a pss bank.
    return (1, 5, 11) if p in (2, 3) else ()


def ps_route_set(p):
    # jj whose scores land in the ps bank pair (all DVE fast-exp here; an
    # ACT-read ps pair costs two half-tile activations and the ps-pair
    # score matmuls pair less cleanly than pss ones — measured slower)
    return ps_fexp_set(p)


def jj_batches(p):
    # score-matmul batches: all jj of a batch are emitted back-to-back so
    # the PE enters/leaves the row-grouped mode once per batch (~200ns per
    # switch pair). 3-jj batches via the ps banks measured slower (worse
    # score-matmul pairing), so everything runs 2-jj batches.
    return tuple((j, j + 1) for j in range(0, JT, 2))


_CACHE: dict = {}


def _emit(tc, x_d, w_d, bqk_d, bv_d, out_d):
    import concourse.bass as bass
    from concourse import mybir
    from contextlib import ExitStack

    nc = tc.nc
    f32 = mybir.dt.float32
    bf16 = mybir.dt.bfloat16
    i16 = mybir.dt.int16
    Exp = mybir.ActivationFunctionType.Exp
    MULT = mybir.AluOpType.mult
    ADD = mybir.AluOpType.add

    E_BUFS = 22
    ctx = ExitStack()
    pers = ctx.enter_context(tc.tile_pool(name="pers", bufs=1))
    qk_pool = ctx.enter_context(tc.tile_pool(name="qk", bufs=PAIRS))
    e_pool = ctx.enter_context(tc.tile_pool(name="epool", bufs=E_BUFS))
    e16_pool = ctx.enter_context(tc.tile_pool(name="e16pool", bufs=8))
    o_pool = ctx.enter_context(tc.tile_pool(name="opool", bufs=4))
    ot_pool = ctx.enter_context(tc.tile_pool(name="otpool", bufs=4))
    sm_pool = ctx.enter_context(tc.tile_pool(name="smpool", bufs=4))
    dr_pool = ctx.enter_context(tc.tile_pool(name="drpool", bufs=4, space="DRAM"))
    # PSUM: 8 banks = ps 2 (proj) + pss 2x2 (scores/exp staging) + po 2 (AV).
    ps_pool = ctx.enter_context(tc.tile_pool(name="pspool", bufs=2, space="PSUM"))
    pss_pool = ctx.enter_context(tc.tile_pool(name="psspool", bufs=2, space="PSUM"))
    po_pool = ctx.enter_context(tc.tile_pool(name="popool", bufs=2, space="PSUM"))

    # ---- ACT exp-table preload: runs during the input DMA phase ----
    warm = sm_pool.tile([1, 8], f32, tag="warm")
    nc.vector.memset(warm, 0.0)
    nc.scalar.activation(warm, warm, Exp)

    # ---- staged input DMA: x group 0 + biases first, then the rest ----
    xt_sb = pers.tile([128, KT, T], bf16)  # xT[k, tok] per k-tile
    bqk_sb = pers.tile([128, PAIRS, 2], f32)  # [part, pair, q/k] per-partition bias
    # x group 0 per k-tile on the sync queue (startup chains drain at DMA
    # arrival pace); groups 1-3 as one coalesced 3D DMA each; bqk alone on
    # the scalar queue (each scalar-queue issue costs ~600ns of ACT time).
    nc.scalar.dma_start(bqk_sb, bqk_d)
    # group 0 per k-tile (startup chains drain at per-tile arrival pace);
    # groups 1-3 in half-group chunks: 6 issues instead of 24 drains the
    # sync issue queue ~15us earlier, so the k(g2)/k(g3) chains in the
    # pair-0 weave never wait on x arrival.
    for kk in range(KT):
        nc.sync.dma_start(
            xt_sb[:, kk, 0:512], x_d[kk * 128 : (kk + 1) * 128, 0:512]
        )
    x_r = x_d.rearrange("(kk p) t -> p kk t", p=128)
    for g in range(1, IG):
        for half in range(2):
            k0 = half * 4
            nc.sync.dma_start(
                xt_sb[:, k0 : k0 + 4, g * 512 : (g + 1) * 512],
                x_r[:, k0 : k0 + 4, g * 512 : (g + 1) * 512],
            )

    # w: pair-0 qk columns + v-bias + v columns now; the remaining qk columns
    # (pairs 1-3, 1.5MB) are issued from the weave once the x stream is done,
    # so they never compete with x for HBM bandwidth.
    w_sb = pers.tile([128, KT, WCOLS], bf16)
    bv_bc = pers.tile([128, VCOLS], f32)  # v-bias broadcast across token rows
    for kk in range(KT):
        nc.gpsimd.dma_start(w_sb[:, kk, 0:256], w_d[kk * 128 : (kk + 1) * 128, 0:256])
    nc.gpsimd.dma_start(bv_bc, bv_d.partition_broadcast(128))
    # v columns on the scalar queue: it is idle after bqk (ScalarE's first
    # exp is not until ~14us) and the gpsimd queue is busy issuing the
    # qk columns — this lands v ~5us earlier so the first AVs are not
    # gated on the v projection.
    for kk in range(KT):
        nc.scalar.dma_start(
            w_sb[:, kk, 1024:WCOLS], w_d[kk * 128 : (kk + 1) * 128, 1024:WCOLS]
        )

    def w_rest_dma_units():
        # sync queue: idle once the x stream is done, and ScalarE/Pool now
        # carry exp work so their queues must stay clear.
        def half(lo_kk):
            def emit():
                for kk in range(lo_kk, lo_kk + 4):
                    nc.sync.dma_start(
                        w_sb[:, kk, 256:1024],
                        w_d[kk * 128 : (kk + 1) * 128, 256:1024],
                    )

            return emit

        return [half(0), half(4)]

    # ---- v ones-columns ----
    v_sb = pers.tile([128, JT, NH_LOCAL, HS + 1], bf16)
    nc.vector.memset(v_sb[:, :, :, HS : HS + 1], 1.0)
    # ones row for the tail-norm partition broadcast (ones^T @ rcp on PE);
    # bf16 so the broadcast matmul streams 1 row/cycle (1.0 and 1/den are
    # fine in bf16 here: it scales only the final ig's normalization)
    ones_sb = pers.tile([1, HS], bf16)
    nc.vector.memset(ones_sb, 1.0)

    # ================= emission engine =================
    # Work units are closures emitting ~2 matmuls (or a small DVE op) each.
    # They are pumped into the per-jj slack of the attention loop.

    def qk_chain_units(p, g, qk):
        # q^T or k^T for pair p, token group g -> qk_tiles[p][:, qk, g*512:]
        # psum partitions 0-63 <- head 2p, 64-127 <- head 2p+1
        st = {}

        def quarter(q4):
            def emit():
                if q4 == 0:
                    st["ps"] = ps_pool.tile([128, 512], f32, tag="ps", name="psqk")
                ps = st["ps"]
                c0 = (p * 2 + qk) * 128
                for kk in range(2 * q4, 2 * q4 + 2):
                    nc.tensor.matmul(
                        ps,
                        w_sb[:, kk, c0 : c0 + 128],
                        xt_sb[:, kk, g * 512 : (g + 1) * 512],
                        start=(kk == 0),
                        stop=(kk == KT - 1),
                    )
                if q4 == 3:
                    nc.vector.tensor_scalar_add(
                        qk_tiles[p][:, qk, g * 512 : (g + 1) * 512],
                        ps,
                        bqk_sb[:, p, qk : qk + 1],
                    )

            return emit

        return [quarter(q4) for q4 in range(4)]

    def v_chain_units(tt):
        # v[tok tile tt, all heads] = xT.T @ Wv, plus folded v-bias
        st = {}

        def quarter(q4):
            def emit():
                if q4 == 0:
                    st["ps"] = ps_pool.tile([128, 512], f32, tag="ps", name="psv")
                ps = st["ps"]
                for kk in range(2 * q4, 2 * q4 + 2):
                    nc.tensor.matmul(
                        ps,
                        xt_sb[:, kk, tt * 128 : (tt + 1) * 128],
                        w_sb[:, kk, 1024:1536],
                        start=(kk == 0),
                        stop=(kk == KT - 1),
                    )
                if q4 == 3:
                    nc.vector.tensor_tensor(
                        v_sb[:, tt, :, 0:HS],
                        ps.rearrange("p (h c) -> p h c", c=HS),
                        bv_bc.rearrange("p (h c) -> p h c", c=HS),
                        op=ADD,
                    )
                    v_emitted[tt] = True

            return emit

        return [quarter(q4) for q4 in range(4)]

    qk_tiles = [
        qk_pool.tile([128, 2, T], bf16, tag="qkt", name=f"qkt{_p}") for _p in range(PAIRS)
    ]
    v_emitted = [False] * JT

    # Global AV queue: (slot, p, ig, jj, e_ap) emitted strictly in order, each
    # gated on its v tile having been emitted AND at least one slot old (so
    # the exp latency never sits between consecutive scores in the PE stream).
    av_queue = []
    norm_jobs = []  # normalization pipeline jobs (step >= 1; step 0 runs inline)
    po_live = {}
    slot_ctr = [0]

    def pump_avs(limit=3, flush=False):
        n = 0
        while av_queue and n < limit:
            sl, p, ig, jj, e_bf = av_queue[0]
            if not v_emitted[jj]:
                break
            # two slots of delay for every tile: with scores batched two jj
            # at a time, AV(jj+1) precedes the next scores batch in the
            # in-order PE stream, and its exp lands ~2.3us after the batch
            # (two back-to-back ACT exps); one slot of slack is not enough
            # and the exp latency would pace the whole PE stream. Fast-exp
            # tiles get one more slot: the DVE runs behind a deeper queue.
            delay = 3 if (jj in pss_fexp_set(p) or jj in ps_fexp_set(p)) else 2
            if not flush and sl + delay > slot_ctr[0]:
                break
            if (p, ig) not in po_live:
                po_live[(p, ig)] = [
                    po_pool.tile([65, 512], f32, tag="po", name=f"po{_h}")
                    for _h in range(2)
                ]
            po = po_live[(p, ig)]
            av_queue.pop(0)
            for h in range(2):
                nc.tensor.matmul(
                    po[h],
                    v_sb[:, jj, 2 * p + h, :],
                    e_bf[:, h * 512 : (h + 1) * 512],
                    start=(jj == 0),
                    stop=(jj == JT - 1),
                )
            n += 1
            if jj == JT - 1:
                # Free the po bank pair NOW (copy to SBUF) so the next ig's
                # AVs — emitted later — see the reader before the reuse.
                po = po_live.pop((p, ig))
                o_t = [
                    o_pool.tile([65, 512], f32, tag="o", name=f"ot{_h}")
                    for _h in range(2)
                ]
                # the next phase's first AV reuses these po banks: in the
                # interleaved pair-2/3 phases split the two copies across
                # ScalarE and VectorE so both banks free ~0.7us sooner
                if p >= 2:
                    nc.scalar.activation(
                        o_t[0], po[0], mybir.ActivationFunctionType.Copy
                    )
                    nc.vector.tensor_copy(o_t[1], po[1])
                else:
                    for h in range(2):
                        nc.vector.tensor_copy(o_t[h], po[h])
                norm_jobs.append({"p": p, "ig": ig, "o": o_t, "step": 1})

    # Normalization pipeline: remaining steps spread across subsequent slots
    # so the DRAM-bounce latency never blocks the in-order DVE stream.
    def norm_tail_fast(job):
        # Final (p,ig): PE is idle, so broadcast 1/den across partitions with
        # a tiny ones^T @ rcp matmul instead of the 3-DMA DRAM bounce.
        p, ig = job["p"], job["ig"]
        den_b = [
            sm_pool.tile([1, 512], bf16, tag="rcp", name=f"denb{_h}")
            for _h in range(2)
        ]
        bc = [
            po_pool.tile([64, 512], f32, tag="po", name=f"bc{_h}") for _h in range(2)
        ]
        rb = [
            sm_pool.tile([64, 512], f32, tag="denbc", name=f"rb{_h}")
            for _h in range(2)
        ]
        ot2 = ot_pool.tile([128, 512], f32, tag="ot")
        for h in range(2):
            # broadcast den (not 1/den) across partitions — the reciprocal
            # then runs on a [64,512] tile where the DVE is fast, instead of
            # on a single-partition row (~6.5ns/elem, 3.3us)
            nc.vector.tensor_copy(den_b[h], job["o"][h][64:65, :])
            nc.tensor.matmul(bc[h], ones_sb, den_b[h], start=True, stop=True)
            # full RECIPROCAL is ~6.5ns/elem (3.3us here); the NR-seeded
            # approx (18 bits; den is a benign ~1e2..1e3 positive value)
            # runs at ~1 elem/lane/cycle on this multi-partition tile
            nc.vector.reciprocal_approx_fast(rb[h], bc[h])
            nc.vector.tensor_tensor(
                ot2[h * 64 : (h + 1) * 64, :], job["o"][h][0:64, :], rb[h], op=MULT
            )
        nc.sync.dma_start(
            out_d[128 * p : 128 * (p + 1), ig * 512 : (ig + 1) * 512], ot2
        )

    def norm_step():
        if not norm_jobs:
            return
        job = norm_jobs[0]
        p, ig, step = job["p"], job["ig"], job["step"]
        if step == 1 and (p, ig) == (PAIRS - 1, IG - 1):
            norm_tail_fast(job)
            norm_jobs.pop(0)
            return
        if step == 1:
            # den row -> [128, 4] transposed layout (SBUF->SBUF DMA)
            rct = sm_pool.tile([128, 8], f32, tag="rct")
            for h in range(2):
                nc.sync.dma_start(rct[:, h * 4 : (h + 1) * 4], job["o"][h][64:65, :])
            job["rct"] = rct
        elif step == 0:
            raise AssertionError("step 0 runs inline in pump_avs")
        elif step == 2:
            rcp = sm_pool.tile([128, 8], f32, tag="rcp")
            nc.vector.reciprocal(rcp, job["rct"])
            job["rcp"] = rcp
        elif step == 3:
            rcd = dr_pool.tile([2, 512], f32, tag="rcd")
            for h in range(2):
                nc.sync.dma_start(rcd[h], job["rcp"][:, h * 4 : (h + 1) * 4])
            job["rcd"] = rcd
        elif step == 4:
            job["dbc"] = []
            for h in range(2):
                den_bc = sm_pool.tile([64, 512], f32, tag="denbc", name=f"dbc{h}")
                nc.gpsimd.dma_start(den_bc, job["rcd"][h].partition_broadcast(64))
                job["dbc"].append(den_bc)
        elif step == 5:
            # all-SBUF op: runs on GpSimd to keep the VectorE queue clear
            # for the fast-exp tiles that pace the pss recycle
            ot2 = ot_pool.tile([128, 512], f32, tag="ot")
            for h in range(2):
                nc.gpsimd.tensor_tensor(
                    ot2[h * 64 : (h + 1) * 64, :],
                    job["o"][h][0:64, :],
                    job["dbc"][h],
                    op=MULT,
                )
            nc.sync.dma_start(
                out_d[128 * p : 128 * (p + 1), ig * 512 : (ig + 1) * 512], ot2
            )
            norm_jobs.pop(0)
            return
        job["step"] += 1

    # ---- startup: q/k chains for pair 0, group 0, interleaved at k-tile
    # granularity so each matmul runs as soon as its x k-tile DMA lands ----
    ps_q = ps_pool.tile([128, 512], f32, tag="ps", name="psq0")
    ps_k = ps_pool.tile([128, 512], f32, tag="ps", name="psk0")
    for kk in range(KT):
        for qk, ps in ((0, ps_q), (1, ps_k)):
            nc.tensor.matmul(
                ps,
                w_sb[:, kk, qk * 128 : (qk + 1) * 128],
                xt_sb[:, kk, 0:512],
                start=(kk == 0),
                stop=(kk == KT - 1),
            )
    for qk, ps in ((0, ps_q), (1, ps_k)):
        nc.vector.tensor_scalar_add(
            qk_tiles[0][:, qk, 0:512], ps, bqk_sb[:, 0, qk : qk + 1]
        )

    # ---- build weave schedules ----
    # pair 0: k chains first (hard scores deadlines at jj=4g; x groups arrive
    # progressively so these are also arrival-ordered), then q(g1), then the
    # v chains (AVs defer via the queue), with q(g2)/q(g3) at the end (their
    # deadlines are late). The w remainder DMA rides early in the stream,
    # once x is (nearly) done. Later pairs: chains prefetched in prior phase.
    pair_units = {p: [] for p in range(PAIRS)}
    u0 = pair_units[0]
    # k chains are hard in-order deadlines (scores jj=4g at slot 4g) AND
    # gate on x-group arrival, so each is placed at the slot its group
    # lands; v chains fill between (their AVs defer via the queue and the
    # deep e pool absorbs the backlog); q(g1) before ig1.
    u0.extend(qk_chain_units(0, 1, 1))  # k(g1): x g1 lands ~slot 1
    for tt in range(2):
        u0.extend(v_chain_units(tt))
    u0.extend(qk_chain_units(0, 2, 1))  # k(g2): x g2 lands ~slot 3-4
    for tt in range(2, 4):
        u0.extend(v_chain_units(tt))
    u0.extend(qk_chain_units(0, 3, 1))  # k(g3): x g3 lands ~slot 6-7
    u0.extend(qk_chain_units(0, 1, 0))  # q(g1) before ig1
    for tt in range(4, 10):
        u0.extend(v_chain_units(tt))
    u0.extend(w_rest_dma_units())  # issue pairs 1-3 W columns (HBM now free)
    for tt in range(10, JT):
        u0.extend(v_chain_units(tt))
    u0.extend(qk_chain_units(0, 2, 0))  # q(g2) before ig2
    u0.extend(qk_chain_units(0, 3, 0))  # q(g3) before ig3
    for p in range(1, PAIRS):
        for g in range(IG):
            for qk in range(2):
                pair_units[p].extend(qk_chain_units(p, g, qk))

    # interleave: during each ig-phase, pump the leftover units of its own
    # schedule (front-loaded) plus the next schedule's prefetch units
    # (~1 matmul/slot). jj are processed in batches of 2 in all phases so
    # the PE switches between row-grouped score matmuls and full-array
    # AV/proj matmuls half as often (each switch costs ~100ns of array
    # drain). Pairs 2 and 3 run as interleaved ig-phases — (2,0),(3,0),
    # (2,1),(3,1),... — so their combined exp demand is smoothed across
    # ScalarE/VectorE instead of pair 3 (no chain filler, lightest PE)
    # being exp-bound on its own.
    def attention_phase(p, ig, own, nxt):
        pss_fexp = pss_fexp_set(p)
        ps_fexp = ps_fexp_set(p)
        ps_route = ps_route_set(p)
        qk_t = qk_tiles[p]

        def scores_mm(dst, jj, h):
            base = 64 * h
            nc.tensor.matmul(
                dst,
                qk_t[base : base + 64, 1, jj * 128 : (jj + 1) * 128],
                qk_t[base : base + 64, 0, ig * 512 : (ig + 1) * 512],
                start=True,
                stop=True,
            )

        def emit_scores(jj):
            # scores^T for both heads (row-tiled concurrent pair)
            pss = pss_pool.tile([128, 1024], f32, tag="pss")
            for h in range(2):
                scores_mm(pss[:, h * 512 : (h + 1) * 512], jj, h)
            return pss

        for batch in jj_batches(p):
            # emit the whole batch's score matmuls back-to-back, then the
            # exp ops in jj order
            staged = []
            for j2 in batch:
                if j2 in ps_route:
                    psf = [
                        ps_pool.tile([128, 512], f32, tag="ps", name=f"psf{_h}")
                        for _h in range(2)
                    ]
                    for h in range(2):
                        scores_mm(psf[h], j2, h)
                    staged.append(("f" if j2 in ps_fexp else "pa", j2, psf))
                else:
                    staged.append(
                        ("v" if j2 in pss_fexp else "a", j2, emit_scores(j2))
                    )
            for o in staged:
                cls, j2, src = o
                if cls == "a":
                    e_t = e_pool.tile([128, 1024], bf16, tag="e")
                    nc.scalar.activation(e_t, src, Exp, scale=0.125)
                    av_queue.append((slot_ctr[0], p, ig, j2, e_t))
                elif cls == "pa":
                    # ps-routed scores exp'd on ScalarE as two half tiles
                    e_t = e_pool.tile([128, 1024], bf16, tag="e")
                    for h in range(2):
                        nc.scalar.activation(
                            e_t[:, h * 512 : (h + 1) * 512], src[h], Exp, scale=0.125
                        )
                    av_queue.append((slot_ctr[0], p, ig, j2, e_t))
                elif cls == "v":
                    # single [128,1024] op straight off the pss bank
                    e16 = e16_pool.tile([128, 1024], i16, tag="e16")
                    nc.vector.tensor_scalar(
                        e16, src, FEXP_A, FEXP_B, op0=MULT, op1=ADD
                    )
                    av_queue.append((slot_ctr[0], p, ig, j2, e16.bitcast(bf16)))
                else:
                    e16 = e16_pool.tile([128, 1024], i16, tag="e16")
                    for h in range(2):
                        nc.vector.tensor_scalar(
                            e16[:, h * 512 : (h + 1) * 512],
                            src[h],
                            FEXP_A,
                            FEXP_B,
                            op0=MULT,
                            op1=ADD,
                        )
                    av_queue.append((slot_ctr[0], p, ig, j2, e16.bitcast(bf16)))
            n_f = sum(
                1
                for a in av_queue
                if a[3] in pss_fexp_set(a[1]) or a[3] in ps_fexp_set(a[1])
            )
            assert n_f <= 6, "e16 backlog exceeds pool depth"
            for jj in batch:
                # emission-order safety: a pool slot must not be re-tiled
                # before its deferred AV reader has been emitted
                assert len(av_queue) <= E_BUFS - 2, (
                    f"AV backlog {len(av_queue)} exceeds e pool depth"
                )
                # weave proj units into the slack of this slot; next-schedule
                # prefetch is paced at ~1 unit (2 matmuls) per slot
                if own:
                    for _ in range(min(3, len(own))):
                        own.pop(0)()
                elif nxt:
                    nxt.pop(0)()
                pump_avs(limit=3)
                norm_step()
                slot_ctr[0] += 1

    # pairs 2 and 3 share one prefetch schedule (built during pair 1) and
    # run as interleaved ig-phases
    merged23 = [u for ab in zip(pair_units[2], pair_units[3]) for u in ab]
    phases = (
        [(0, g) for g in range(IG)]
        + [(1, g) for g in range(IG)]
        + [(q, g) for g in range(IG) for q in (2, 3)]
    )
    for p, ig in phases:
        if p == 0:
            own, nxt = pair_units[0], pair_units[1]
        elif p == 1:
            own, nxt = pair_units[1], merged23
        else:
            own, nxt = merged23, []
        attention_phase(p, ig, own, nxt)
    # flush stragglers
    while av_queue:
        pump_avs(flush=True)
    while norm_jobs:
        norm_step()
    ctx.close()


def _build():
    import concourse.tile as tile
    from concourse import bacc, mybir

    f32 = mybir.dt.float32
    nc = bacc.Bacc("TRN2", target_bir_lowering=False, debug=False, num_devices=8)
    x_d = nc.dram_tensor("x", [D, T], mybir.dt.bfloat16, kind="ExternalInput").ap()
    w_d = nc.dram_tensor("w", [D, WCOLS], mybir.dt.bfloat16, kind="ExternalInput").ap()
    bqk_d = nc.dram_tensor("bqk", [128, PAIRS, 2], f32, kind="ExternalInput").ap()
    bv_d = nc.dram_tensor("bv", [VCOLS], f32, kind="ExternalInput").ap()
    out_d = nc.dram_tensor("out", [VCOLS, T], f32, kind="ExternalOutput").ap()
    with tile.TileContext(nc) as tc:
        _emit(tc, x_d, w_d, bqk_d, bv_d, out_d)
    nc.compile()
    return nc


def get_nc():
    if "nc" not in _CACHE:
        _CACHE["nc"] = _build()
    return _CACHE["nc"]


def make_in_maps(x, W_qkv, b_qkv):
    """Shard full inputs into 8 per-core input maps."""
    x = np.asarray(x, dtype=np.float32)
    W_qkv = np.asarray(W_qkv, dtype=np.float32)
    b_qkv = np.asarray(b_qkv, dtype=np.float32)
    in_maps = []
    for c in range(8):
        b, half = divmod(c, 2)
        w_c = W_qkv[:, half * WCOLS : (half + 1) * WCOLS]
        b_c = b_qkv[half * WCOLS : (half + 1) * WCOLS]
        # permute columns: paired q/k blocks first, then v cols in head order
        w3 = w_c.reshape(D, NH_LOCAL, 3, HS)
        blocks = []
        for p in range(PAIRS):
            for qk in range(2):
                blocks.append(w3[:, 2 * p, qk, :])
                blocks.append(w3[:, 2 * p + 1, qk, :])
        for h in range(NH_LOCAL):
            blocks.append(w3[:, h, 2, :])
        w_c = np.concatenate(blocks, axis=1).astype(ml_dtypes.bfloat16)
        # per-partition qk bias: partitions 0-63 <- head 2p, 64-127 <- head 2p+1
        bqk = np.zeros((128, PAIRS, 2), dtype=np.float32)
        for p in range(PAIRS):
            for qk in range(2):
                bqk[0:64, p, qk] = b_c[(2 * p) * 192 + qk * 64 : (2 * p) * 192 + (qk + 1) * 64]
                bqk[64:128, p, qk] = b_c[(2 * p + 1) * 192 + qk * 64 : (2 * p + 1) * 192 + (qk + 1) * 64]
        bv = np.ascontiguousarray(
            b_c.reshape(NH_LOCAL, 3, HS)[:, 2, :].reshape(VCOLS)
        )
        in_maps.append(
            {
                "x": np.ascontiguousarray(x[b].T).astype(ml_dtypes.bfloat16),
                "w": w_c,
                "bqk": bqk,
                "bv": bv,
            }
        )
    return in_maps


def assemble_output(results):
    out = np.zeros((4, T, D), dtype=np.float32)
    for c in range(8):
        b, half = divmod(c, 2)
        out[b, :, half * VCOLS : (half + 1) * VCOLS] = results[c]["out"].T
    return out


def kernel(x, W_qkv, b_qkv):
    from concourse.bass_utils import run_bass_kernel_spmd

    nc = get_nc()
    in_maps = make_in_maps(x, W_qkv, b_qkv)
    res = run_bass_kernel_spmd(nc, in_maps, core_ids=list(range(8)))
    return assemble_output(res.results)


if __name__ == "__main__":
    xs = np.random.randn(4, T, D).astype(np.float32)
    Ws = (np.random.randn(D, 3 * D) / 32.0).astype(np.float32)
    bs = (np.random.randn(3 * D) * 0.02).astype(np.float32)
    o = kernel(xs, Ws, bs)
    print(o.shape, o.dtype)
